# revision 3
# baseline (speedup 1.0000x reference)
"""Trainium2 Bass kernel for nn_ClusterPolicyNetwork.

Computation (reference.py):
  1. 4-head self-attention over N=1024 nodes (D=128), returns attended nodes
     and head-averaged attention map attn_w [1,1024,1024].
  2. Pairwise matching MLP over (task, node) grid:
     h1 = relu(t1[t] + n1[n] + b1)  (256)
     h2 = relu(W2 h1 + b2)          (128)
     score = sigmoid(W3 h2 + b3)    -> [1024, 1024]
  3. Coordination head on mean of attended -> [1, 32].

Sharding: tasks and attention q-rows both split 128/core across 8 cores.
Each core computes attention for its q-slice; an AllGather shares the
attended nodes; each core then computes a [128, 1024] score tile.

Layouts are "transposed" (feature on partitions) throughout. All biases are
folded into evacuation ops or host-precomputed weight/bias transforms.
bf16 is used for matmul operands (fp32 accumulation everywhere).
"""

import math

import numpy as np
import ml_dtypes

from concourse import mybir
import concourse.bacc as bacc
import concourse.tile as tile
from concourse.bass_utils import run_bass_kernel_spmd

F32 = mybir.dt.float32
BF16 = mybir.dt.bfloat16
AF = mybir.ActivationFunctionType
OP = mybir.AluOpType

NCORES = 8
N = 1024          # nodes
T = 1024          # tasks
D = 128           # d_model
H = 4             # heads
DH = 32           # head dim
TLOC = T // NCORES
QLOC = N // NCORES
GROUPS = TLOC // 4

_CACHE = {}


def _build_program():
    nc = bacc.Bacc("TRN2", target_bir_lowering=False, debug=False,
                   enable_asserts=True, num_devices=NCORES)

    def din(name, shape, dt=BF16):
        return nc.dram_tensor(name, shape, dt, kind="ExternalInput").ap()

    def dout(name, shape, dt=F32):
        return nc.dram_tensor(name, shape, dt, kind="ExternalOutput").ap()

    # inputs (host-prepped layouts)
    xT_d = din("xT", [D, N])                 # node_states[0].T      (shared)
    xTq_d = din("xTq", [D, QLOC])            # q-slice of xT         (per core)
    taskT_d = din("taskT", [D, TLOC])        # task slice, transposed (per core)
    wqT_d = din("wqT", [D, D])
    wkT_d = din("wkT", [D, D])
    wvT_d = din("wvT", [D, D])
    woT4_d = din("woT4", [D, D])             # out_w.T * 4
    w1aT_d = din("w1aT", [D, 256])
    w1bT_d = din("w1bT", [D, 256])
    w2Ta_d = din("w2Ta", [128, 128])         # W2[:, :128].T
    w2Tb_d = din("w2Tb", [128, 128])         # W2[:, 128:].T
    w3c_d = din("w3c", [128, 1])
    wc1T_d = din("wc1T", [D, 256])           # Wc1.T / 1024
    wc2Ta_d = din("wc2Ta", [128, 32])
    wc2Tb_d = din("wc2Tb", [128, 32])
    ident_d = din("ident", [128, 128], F32)
    bqs_d = din("bqs", [128, 1], F32)        # in_proj_b[:128]/sqrt(32)
    bk_d = din("bk", [128, 1], F32)
    bc1f_d = din("bc1f", [128, 2], F32)      # W1b@bprime + b1  (2 chunks)
    b2c_d = din("b2c", [128, 1], F32)
    b3c_d = din("b3c", [128, 1], F32)
    bcc1_d = din("bcc1", [128, 2], F32)      # Wc1@bprime + bc1 (2 chunks)
    bcc2_d = din("bcc2", [32, 1], F32)

    scores_d = dout("scores", [TLOC, N])
    attn_d = dout("attn", [QLOC, N])
    logits_d = dout("logits", [32, 1])

    SQS = 1.0 / math.sqrt(DH)

    with tile.TileContext(nc) as tc:
        with (
            tc.tile_pool(name="wpool", bufs=1) as wp,
            tc.tile_pool(name="sbA", bufs=1) as sa,
            tc.tile_pool(name="dramp", bufs=1, space="DRAM") as dp,
        ):
            # --- load constants ---
            xT = wp.tile([D, N], BF16)
            nc.sync.dma_start(xT[:], xT_d)
            xTq = wp.tile([D, QLOC], BF16)
            nc.sync.dma_start(xTq[:], xTq_d)
            taskT = wp.tile([D, TLOC], BF16)
            nc.sync.dma_start(taskT[:], taskT_d)
            wqT = wp.tile([D, D], BF16)
            nc.sync.dma_start(wqT[:], wqT_d)
            wkT = wp.tile([D, D], BF16)
            nc.sync.dma_start(wkT[:], wkT_d)
            wvT = wp.tile([D, D], BF16)
            nc.sync.dma_start(wvT[:], wvT_d)
            woT4 = wp.tile([D, D], BF16)
            nc.sync.dma_start(woT4[:], woT4_d)
            w1aT = wp.tile([D, 256], BF16)
            nc.sync.dma_start(w1aT[:], w1aT_d)
            w1bT = wp.tile([D, 256], BF16)
            nc.sync.dma_start(w1bT[:], w1bT_d)
            w2Ta = wp.tile([128, 128], BF16)
            nc.sync.dma_start(w2Ta[:], w2Ta_d)
            w2Tb = wp.tile([128, 128], BF16)
            nc.sync.dma_start(w2Tb[:], w2Tb_d)
            w3c = wp.tile([128, 1], BF16)
            nc.sync.dma_start(w3c[:], w3c_d)
            wc1T = wp.tile([D, 256], BF16)
            nc.sync.dma_start(wc1T[:], wc1T_d)
            wc2Ta = wp.tile([128, 32], BF16)
            nc.sync.dma_start(wc2Ta[:], wc2Ta_d)
            wc2Tb = wp.tile([128, 32], BF16)
            nc.sync.dma_start(wc2Tb[:], wc2Tb_d)
            ident = wp.tile([128, 128], F32)
            nc.sync.dma_start(ident[:], ident_d)
            bqs = wp.tile([128, 1], F32)
            nc.sync.dma_start(bqs[:], bqs_d)
            bk = wp.tile([128, 1], F32)
            nc.sync.dma_start(bk[:], bk_d)
            bc1f = wp.tile([128, 2], F32)
            nc.sync.dma_start(bc1f[:], bc1f_d)
            b2c = wp.tile([128, 1], F32)
            nc.sync.dma_start(b2c[:], b2c_d)
            b3c = wp.tile([128, 1], F32)
            nc.sync.dma_start(b3c[:], b3c_d)
            bcc1 = wp.tile([128, 2], F32)
            nc.sync.dma_start(bcc1[:], bcc1_d)
            bcc2 = wp.tile([32, 1], F32)
            nc.sync.dma_start(bcc2[:], bcc2_d)

            # ---------- phase A: attention on the q-slice ----------
            qT_bf = sa.tile([D, QLOC], BF16)
            kT_bf = sa.tile([D, N], BF16)
            v_nm = sa.tile([128, N], BF16)     # [n_in_block, 128*b + d']
            t1b = sa.tile([128, 2 * TLOC], F32)  # per-task bias cols (2 chunks)

            with tc.tile_pool(name="psA", bufs=2, space="PSUM") as psA:
                qT_ps = psA.tile([D, QLOC], F32, tag="pa")
                nc.tensor.matmul(qT_ps[:], wqT[:], xTq[:], start=True, stop=True)
                nc.scalar.activation(qT_bf[:], qT_ps[:], AF.Identity,
                                     bias=bqs[:, 0:1], scale=SQS)

                kT_ps = psA.tile([D, N], F32, tag="pa")
                for nh in range(2):
                    nc.tensor.matmul(kT_ps[:, 512 * nh:512 * nh + 512], wkT[:],
                                     xT[:, 512 * nh:512 * nh + 512],
                                     start=True, stop=True)
                nc.scalar.activation(kT_bf[:], kT_ps[:], AF.Identity,
                                     bias=bk[:, 0:1])

                v_ps = psA.tile([128, N], F32, tag="pa")
                for b in range(8):
                    nc.tensor.matmul(v_ps[:, 128 * b:128 * b + 128],
                                     xT[:, 128 * b:128 * b + 128], wvT[:],
                                     start=True, stop=True)
                nc.vector.tensor_copy(v_nm[:], v_ps[:])

                t1_ps = psA.tile([128, 2 * TLOC], F32, tag="pa")
                for ch in range(2):
                    nc.tensor.matmul(t1_ps[:, TLOC * ch:TLOC * ch + TLOC],
                                     w1aT[:, 128 * ch:128 * ch + 128], taskT[:],
                                     start=True, stop=True)
                nc.scalar.copy(t1b[:], t1_ps[:])

            # softmax (no max subtraction: logits are O(1))
            attn_q4 = [sa.tile([QLOC, N], F32, name=f"attn_q4_{h}") for h in range(H)]
            with tc.tile_pool(name="psS", bufs=3, space="PSUM") as psS:
                for h in range(H):
                    s_ps = psS.tile([QLOC, N], F32, tag="s")
                    for kh in range(2):
                        nc.tensor.matmul(s_ps[:, 512 * kh:512 * kh + 512],
                                         qT_bf[32 * h:32 * h + 32, :],
                                         kT_bf[32 * h:32 * h + 32,
                                               512 * kh:512 * kh + 512],
                                         start=True, stop=True,
                                         tile_position=(32 * h, 0))
                    exp_h = sa.tile([QLOC, N], F32, tag="exp", bufs=2,
                                    name=f"exp_{h}")
                    sumexp = sa.tile([QLOC, 1], F32, tag="sumexp", bufs=4,
                                     name=f"sumexp_{h}")
                    nc.scalar.activation(exp_h[:], s_ps[:], AF.Exp,
                                         accum_out=sumexp[:])
                    se4 = sa.tile([QLOC, 1], F32, tag="se4", bufs=4,
                                  name=f"se4_{h}")
                    nc.vector.tensor_scalar_mul(se4[:], sumexp[:], 4.0)
                    rec4 = sa.tile([QLOC, 1], F32, tag="rec4", bufs=4,
                                   name=f"rec4_{h}")
                    nc.vector.reciprocal(rec4[:], se4[:])
                    nc.vector.tensor_scalar_mul(attn_q4[h][:], exp_h[:],
                                                rec4[:, 0:1])

            # attn_w output: sum of quarter-normalized heads
            av1 = sa.tile([QLOC, N], F32)
            nc.vector.tensor_tensor(av1[:], attn_q4[0][:], attn_q4[1][:], op=OP.add)
            av2 = sa.tile([QLOC, N], F32)
            nc.vector.tensor_tensor(av2[:], attn_q4[2][:], attn_q4[3][:], op=OP.add)
            avg = sa.tile([QLOC, N], F32)
            nc.vector.tensor_tensor(avg[:], av1[:], av2[:], op=OP.add)
            nc.sync.dma_start(attn_d, avg[:])

            # transpose attn (per 128-node block) and apply attention to v
            attn_T = [sa.tile([128, 4 * QLOC], BF16, name=f"attn_T_{b}")
                      for b in range(8)]
            with tc.tile_pool(name="psT", bufs=3, space="PSUM") as psT:
                for b in range(8):
                    tr_ps = psT.tile([128, 4 * QLOC], F32, tag="tr")
                    for h in range(H):
                        nc.tensor.transpose(tr_ps[:, 128 * h:128 * h + 128],
                                            attn_q4[h][:, 128 * b:128 * b + 128],
                                            ident[:])
                    if b % 2 == 0:
                        nc.scalar.copy(attn_T[b][:], tr_ps[:])
                    else:
                        nc.vector.tensor_copy(attn_T[b][:], tr_ps[:])

            attendedT_c = sa.tile([D, QLOC], F32)
            with tc.tile_pool(name="psC", bufs=1, space="PSUM") as psC:
                ctxT_ps = psC.tile([D, QLOC], F32, tag="ctx")
                for b in range(8):
                    for h in range(H):
                        nc.tensor.matmul(
                            ctxT_ps[32 * h:32 * h + 32, :],
                            v_nm[:, 128 * b + 32 * h:128 * b + 32 * h + 32],
                            attn_T[b][:, 128 * h:128 * h + 128],
                            start=(b == 0), stop=(b == 7),
                            tile_position=(0, 32 * h),
                        )
                ctxT_bf = sa.tile([D, QLOC], BF16)
                nc.scalar.copy(ctxT_bf[:], ctxT_ps[:])

                att_ps = psC.tile([QLOC, D], F32, tag="att")
                nc.tensor.matmul(att_ps[:], ctxT_bf[:], woT4[:],
                                 start=True, stop=True)
                att_sb = sa.tile([QLOC, D], F32)
                nc.scalar.copy(att_sb[:], att_ps[:])
                attT_ps = psC.tile([D, QLOC], F32, tag="attT")
                nc.tensor.transpose(attT_ps[:], att_sb[:], ident[:])
                nc.scalar.copy(attendedT_c[:], attT_ps[:])

            # AllGather attendedT slices -> full attendedT [D, N]
            ag_in = dp.tile([D, QLOC], F32)
            ag_out = dp.tile([N, D], F32)
            nc.sync.dma_start(ag_in[:], attendedT_c[:])
            nc.gpsimd.collective_compute(
                "AllGather", OP.bypass,
                replica_groups=[list(range(NCORES))],
                ins=[ag_in.opt()], outs=[ag_out.opt()],
            )
            attendedT = sa.tile([D, N], F32)
            for c in range(NCORES):
                nc.sync.dma_start(attendedT[:, 128 * c:128 * c + 128],
                                  ag_out[128 * c:128 * c + 128, :])
            attendedT_bf = sa.tile([D, N], BF16)
            nc.vector.tensor_copy(attendedT_bf[:], attendedT[:])

            # n1T = W1b @ attendedT + (W1b@bprime + b1)   [2 chunks of 128 c]
            n1T_bf = [sa.tile([128, N], BF16, name=f"n1T_{ch}") for ch in range(2)]
            with tc.tile_pool(name="psN", bufs=2, space="PSUM") as psN:
                for ch in range(2):
                    n1_ps = psN.tile([128, N], F32, tag="n1")
                    for nh in range(2):
                        nc.tensor.matmul(n1_ps[:, 512 * nh:512 * nh + 512],
                                         w1bT[:, 128 * ch:128 * ch + 128],
                                         attendedT_bf[:, 512 * nh:512 * nh + 512],
                                         start=True, stop=True)
                    nc.scalar.activation(n1T_bf[ch][:], n1_ps[:], AF.Identity,
                                         bias=bc1f[:, ch:ch + 1])

                # coordination head
                gs_sum = sa.tile([D, 1], F32)
                nc.vector.tensor_reduce(gs_sum[:], attendedT[:],
                                        axis=mybir.AxisListType.X, op=OP.add)
                gs_bf = sa.tile([D, 1], BF16)
                nc.vector.tensor_copy(gs_bf[:], gs_sum[:])
                c1_bf = sa.tile([128, 2], BF16)
                for ch in range(2):
                    c1_ps = psN.tile([128, 1], F32, tag="c1")
                    nc.tensor.matmul(c1_ps[:],
                                     wc1T[:, 128 * ch:128 * ch + 128],
                                     gs_bf[:], start=True, stop=True)
                    nc.scalar.activation(c1_bf[:, ch:ch + 1], c1_ps[:], AF.Relu,
                                         bias=bcc1[:, ch:ch + 1])
                lg_ps = psN.tile([32, 1], F32, tag="lg")
                for ch in range(2):
                    nc.tensor.matmul(lg_ps[:], wc2Ta[:] if ch == 0 else wc2Tb[:],
                                     c1_bf[:, ch:ch + 1],
                                     start=(ch == 0), stop=(ch == 1))
                lg_sb = sa.tile([32, 1], F32)
                nc.scalar.activation(lg_sb[:], lg_ps[:], AF.Identity,
                                     bias=bcc2[:, 0:1])
                nc.sync.dma_start(logits_d, lg_sb[:])

            # ---------- phase B: pairwise matching ----------
            with (
                tc.tile_pool(name="sbB", bufs=3) as sb,
                tc.tile_pool(name="psB", bufs=2, space="PSUM") as psB,
                tc.tile_pool(name="psSc", bufs=2, space="PSUM") as psSc,
            ):
                for g in range(GROUPS):
                    sc_ps = psSc.tile([128, N], F32, tag="sc")
                    for j in range(4):
                        t = 4 * g + j
                        # h1 = relu(n1 + t1[t]) in bf16, per 128-channel chunk
                        h1 = [None, None]
                        for ch in range(2):
                            h1[ch] = sb.tile([128, N], BF16, tag=f"h1_{ch}",
                                             name=f"h1_{ch}_{t}")
                            nc.vector.tensor_scalar(
                                h1[ch][:], n1T_bf[ch][:],
                                t1b[:, TLOC * ch + t:TLOC * ch + t + 1], 0.0,
                                op0=OP.add, op1=OP.max)
                        # h2 accumulation over the 2 chunks
                        g_ps = psB.tile([128, N], F32, tag="g")
                        for nh in range(2):
                            nc.tensor.matmul(g_ps[:, 512 * nh:512 * nh + 512],
                                             w2Ta[:],
                                             h1[0][:, 512 * nh:512 * nh + 512],
                                             start=True, stop=False)
                            nc.tensor.matmul(g_ps[:, 512 * nh:512 * nh + 512],
                                             w2Tb[:],
                                             h1[1][:, 512 * nh:512 * nh + 512],
                                             start=False, stop=True)
                        # r = relu(h2 + b2), bf16 (split between DVE and ACT)
                        r = sb.tile([128, N], BF16, tag="r", name=f"r_{t}")
                        if t % 10 < 3:
                            nc.vector.tensor_scalar(r[:], g_ps[:], b2c[:, 0:1],
                                                    0.0, op0=OP.add, op1=OP.max)
                        else:
                            nc.scalar.activation(r[:], g_ps[:], AF.Relu,
                                                 bias=b2c[:, 0:1])
                        # score row: col-tiled M=1 matmul, lands at partition 32j
                        for nh in range(2):
                            nc.tensor.matmul(
                                sc_ps[32 * j:32 * j + 1,
                                      512 * nh:512 * nh + 512],
                                w3c[:], r[:, 512 * nh:512 * nh + 512],
                                start=True, stop=True,
                                tile_position=(0, 32 * j))
                    sig = sb.tile([128, N], F32, tag="sig", bufs=2,
                                  name=f"sig_{g}")
                    nc.scalar.activation(sig[:], sc_ps[:], AF.Sigmoid,
                                         bias=b3c[:, 0:1])
                    nc.sync.dma_start(scores_d[4 * g:4 * g + 4, :],
                                      sig[0:128:32, :])

    nc.compile()
    return nc


def _prep_inputs(inputs):
    bf = ml_dtypes.bfloat16

    def tb(a):
        return np.ascontiguousarray(a).astype(bf)

    def tf(a):
        return np.ascontiguousarray(a.astype(np.float32))

    x = np.asarray(inputs["node_states"], np.float32)[0]        # [N, D]
    task = np.asarray(inputs["task_features"], np.float32)      # [T, D]
    ipw = np.asarray(inputs["in_proj_w"], np.float32)
    ipb = np.asarray(inputs["in_proj_b"], np.float32)
    out_w = np.asarray(inputs["out_w"], np.float32)
    out_b = np.asarray(inputs["out_b"], np.float32)
    W1 = np.asarray(inputs["W1"], np.float32)
    b1 = np.asarray(inputs["b1"], np.float32)
    W2 = np.asarray(inputs["W2"], np.float32)
    b2 = np.asarray(inputs["b2"], np.float32)
    W3 = np.asarray(inputs["W3"], np.float32)
    b3 = np.asarray(inputs["b3"], np.float32)
    Wc1 = np.asarray(inputs["Wc1"], np.float32)
    bc1 = np.asarray(inputs["bc1"], np.float32)
    Wc2 = np.asarray(inputs["Wc2"], np.float32)
    bc2 = np.asarray(inputs["bc2"], np.float32)

    xT = x.T                                                    # [D, N]
    taskT = task.T                                              # [D, T]
    bprime = out_w @ ipb[256:384] + out_b                       # [128]
    bc1f = (W1[:, 128:] @ bprime + b1).reshape(2, 128).T        # [128, 2]
    bcc1 = (Wc1 @ bprime + bc1).reshape(2, 128).T               # [128, 2]

    shared = {
        "xT": tb(xT),
        "wqT": tb(ipw[0:128].T),
        "wkT": tb(ipw[128:256].T),
        "wvT": tb(ipw[256:384].T),
        "woT4": tb(out_w.T * 4.0),
        "w1aT": tb(W1[:, :128].T),
        "w1bT": tb(W1[:, 128:].T),
        "w2Ta": tb(W2[:, :128].T),
        "w2Tb": tb(W2[:, 128:].T),
        "w3c": tb(W3[0][:, None]),
        "wc1T": tb(Wc1.T / 1024.0),
        "wc2Ta": tb(Wc2[:, :128].T),
        "wc2Tb": tb(Wc2[:, 128:].T),
        "ident": tf(np.eye(128, dtype=np.float32)),
        "bqs": tf((ipb[0:128] / math.sqrt(DH))[:, None]),
        "bk": tf(ipb[128:256][:, None]),
        "bc1f": tf(bc1f),
        "b2c": tf(b2[:, None]),
        "b3c": tf(np.full((128, 1), b3[0], np.float32)),
        "bcc1": tf(bcc1),
        "bcc2": tf(bc2[:, None]),
    }
    in_maps = []
    for c in range(NCORES):
        m = dict(shared)
        m["xTq"] = tb(xT[:, QLOC * c:QLOC * c + QLOC])
        m["taskT"] = tb(taskT[:, TLOC * c:TLOC * c + TLOC])
        in_maps.append(m)
    return in_maps


def kernel(**inputs):
    if "nc" not in _CACHE:
        _CACHE["nc"] = _build_program()
    nc = _CACHE["nc"]
    in_maps = _prep_inputs(inputs)
    res = run_bass_kernel_spmd(nc, in_maps, list(range(NCORES))).results

    matching = np.concatenate([res[c]["scores"] for c in range(NCORES)], axis=0)
    attn_w = np.concatenate([res[c]["attn"] for c in range(NCORES)], axis=0)[None]
    logits = res[0]["logits"].reshape(1, 32)
    return (matching.astype(np.float32), logits.astype(np.float32),
            attn_w.astype(np.float32))


# revision 13
# speedup vs baseline: 1.0204x; 1.0204x over previous
"""Trainium2 Bass kernel for nn_ClusterPolicyNetwork.

Computation (reference.py):
  1. 4-head self-attention over N=1024 nodes (D=128), returns attended nodes
     and head-averaged attention map attn_w [1,1024,1024].
  2. Pairwise matching MLP over (task, node) grid:
     h1 = relu(t1[t] + n1[n] + b1)  (256)
     h2 = relu(W2 h1 + b2)          (128)
     score = sigmoid(W3 h2 + b3)    -> [1024, 1024]
  3. Coordination head on mean of attended -> [1, 32].

Sharding: tasks and attention q-rows both split 128/core across 8 cores.
Each core computes attention for its q-slice; an AllGather shares the
attended nodes; each core then computes a [128, 1024] score tile.

Layouts are "transposed" (feature on partitions) throughout. All biases are
folded into evacuation ops or host-precomputed weight/bias transforms.
bf16 matmul operands, fp32 accumulation.

Stage-3 trick: the W3 dot for task t uses a stationary [128, 32] matrix with
W3 in column t%32 and zeros elsewhere, accumulated (start/stop per quadrant
column-group) into one persistent PSUM tile so that task t's score row lands
at partition 32*(t%4) + t//4... (see code: row = 32*quad + col), giving a
single sigmoid evacuation for all 128 tasks.
"""

import math

import numpy as np
import ml_dtypes

from concourse import mybir
import concourse.bacc as bacc
import concourse.tile as tile
from concourse.bass_utils import run_bass_kernel_spmd

F32 = mybir.dt.float32
BF16 = mybir.dt.bfloat16
AF = mybir.ActivationFunctionType
OP = mybir.AluOpType

NCORES = 8
N = 1024          # nodes
T = 1024          # tasks
D = 128           # d_model
H = 4             # heads
DH = 32           # head dim
TLOC = T // NCORES
QLOC = N // NCORES
RUN = 8           # tasks per stage2/stage3 batch (PE mode-switch batching)
WPACK_COLS = 2752  # packed bf16 weights (see wsl() offsets)
BPACK_COLS = 136   # packed f32 bias columns + identity

_CACHE = {}


def _build_program(sim=False):
    # sim=True builds a single-core variant with the collective replaced by
    # local DMAs, for cost-model timeline simulation only.
    nc = bacc.Bacc("TRN2", target_bir_lowering=False, debug=False,
                   enable_asserts=True, num_devices=1 if sim else NCORES)

    def din(name, shape, dt=BF16):
        return nc.dram_tensor(name, shape, dt, kind="ExternalInput").ap()

    def dout(name, shape, dt=F32):
        return nc.dram_tensor(name, shape, dt, kind="ExternalOutput").ap()

    # inputs (host-prepped layouts).  All bf16 weights ride in one packed
    # tensor (single DMA); all f32 bias columns + identity in another.
    xT_d = din("xT", [D, N])                 # node_states[0].T      (shared)
    xTq_d = din("xTq", [D, QLOC])            # q-slice of xT         (per core)
    taskT_d = din("taskT", [D, TLOC])        # task slice, transposed (per core)
    wpack_d = din("wpack", [128, WPACK_COLS])
    bpack_d = din("bpack", [128, BPACK_COLS], F32)
    bcc2_d = din("bcc2", [32, 1], F32)

    scores_d = dout("scores", [TLOC, N])
    attn_d = dout("attn", [QLOC, N])
    logits_d = dout("logits", [32, 1])

    SQS = 1.0 / math.sqrt(DH)

    with tile.TileContext(nc) as tc:
        with (
            tc.tile_pool(name="wpool", bufs=1) as wp,
            tc.tile_pool(name="sbA", bufs=1) as sa,
            tc.tile_pool(name="dramp", bufs=1, space="DRAM") as dp,
        ):
            # --- load constants (few big DMAs, spread over engine queues) ---
            xTq = wp.tile([D, QLOC], BF16)
            nc.sync.dma_start(xTq[:], xTq_d)
            xT = wp.tile([D, N], BF16)
            nc.sync.dma_start(xT[:], xT_d)
            wpack = wp.tile([128, WPACK_COLS], BF16)
            nc.gpsimd.dma_start(wpack[:], wpack_d)
            bpack = wp.tile([128, BPACK_COLS], F32)
            nc.scalar.dma_start(bpack[:], bpack_d)
            taskT = wp.tile([D, TLOC], BF16)
            nc.scalar.dma_start(taskT[:], taskT_d)
            bcc2 = wp.tile([32, 1], F32)
            nc.scalar.dma_start(bcc2[:], bcc2_d)

            def wsl(a, b):
                return wpack[:, a:b]

            wqT = wsl(0, 128)
            wkT = wsl(128, 256)
            wvT = wsl(256, 384)
            woT4 = wsl(384, 512)
            w1aT = wsl(512, 768)
            w1bT = wsl(768, 1024)
            w2Ta = wsl(1024, 1152)
            w2Tb = wsl(1152, 1280)
            wc1T = wsl(1280, 1536)
            wc2Ta = wsl(1536, 1568)
            wc2Tb = wsl(1568, 1600)
            identb = wsl(1600, 1728)
            w3m = wsl(1728, 2752)
            bqs = bpack[:, 0:1]
            bk = bpack[:, 1:2]
            bc1f = bpack[:, 2:4]
            b2c = bpack[:, 4:5]
            b3c = bpack[:, 5:6]
            bcc1 = bpack[:, 6:8]
            identf = bpack[:, 8:136]

            # ---------- phase A: attention on the q-slice ----------
            qT_bf = sa.tile([D, QLOC], BF16)
            kT_bf = sa.tile([D, N], BF16)
            v_nm = sa.tile([128, N], BF16)     # [n_in_block, 128*b + d']
            t1b = sa.tile([128, 2 * TLOC], F32)  # per-task bias cols (2 chunks)

            attn_q4 = [sa.tile([QLOC, N], BF16, name=f"attn_q4_{h}")
                       for h in range(H)]
            with tc.tile_pool(name="psA", bufs=1, space="PSUM") as psA:
                qT_ps = psA.tile([D, QLOC], F32, tag="q")
                nc.tensor.matmul(qT_ps[:], wqT, xTq[:], start=True, stop=True)
                nc.scalar.activation(qT_bf[:], qT_ps[:], AF.Identity,
                                     bias=bqs, scale=SQS)

                kT_ps = psA.tile([D, N], F32, tag="k")
                for nh in range(2):
                    nc.tensor.matmul(kT_ps[:, 512 * nh:512 * nh + 512], wkT,
                                     xT[:, 512 * nh:512 * nh + 512],
                                     start=True, stop=True)
                nc.vector.tensor_scalar(kT_bf[:], kT_ps[:], bk, None,
                                        op0=OP.add)

                # softmax (no max subtraction: logits are O(1))
                for h in range(H):
                    s_ps = psA.tile([QLOC, N], F32, tag="s", bufs=2,
                                    name=f"s_ps_{h}")
                    for kh in range(2):
                        nc.tensor.matmul(s_ps[:, 512 * kh:512 * kh + 512],
                                         qT_bf[32 * h:32 * h + 32, :],
                                         kT_bf[32 * h:32 * h + 32,
                                               512 * kh:512 * kh + 512],
                                         start=True, stop=True,
                                         tile_position=(32 * h, 0))
                    exp_h = sa.tile([QLOC, N], F32, tag="exp", bufs=2,
                                    name=f"exp_{h}")
                    sumexp = sa.tile([QLOC, 1], F32, tag="sumexp", bufs=4,
                                     name=f"sumexp_{h}")
                    nc.scalar.activation(exp_h[:], s_ps[:], AF.Exp,
                                         accum_out=sumexp[:])
                    se4 = sa.tile([QLOC, 1], F32, tag="se4", bufs=4,
                                  name=f"se4_{h}")
                    nc.vector.tensor_scalar_mul(se4[:], sumexp[:], 4.0)
                    rec4 = sa.tile([QLOC, 1], F32, tag="rec4", bufs=4,
                                   name=f"rec4_{h}")
                    nc.vector.reciprocal(rec4[:], se4[:])
                    nc.vector.tensor_scalar_mul(attn_q4[h][:], exp_h[:],
                                                rec4[:, 0:1])

                # v projection (n-major) and t1, reusing the k/q psum slots
                v_ps = psA.tile([128, N], F32, tag="k", name="v_ps")
                for b in range(8):
                    nc.tensor.matmul(v_ps[:, 128 * b:128 * b + 128],
                                     xT[:, 128 * b:128 * b + 128], wvT,
                                     start=True, stop=True)
                nc.vector.tensor_copy(v_nm[:], v_ps[:])

                t1_ps = psA.tile([128, 2 * TLOC], F32, tag="q", name="t1_ps")
                for ch in range(2):
                    nc.tensor.matmul(t1_ps[:, TLOC * ch:TLOC * ch + TLOC],
                                     w1aT[:, 128 * ch:128 * ch + 128], taskT[:],
                                     start=True, stop=True)
                nc.scalar.copy(t1b[:], t1_ps[:])

            # transpose attn (per 128-node block) and apply attention to v
            attn_T = [sa.tile([128, 4 * QLOC], BF16, name=f"attn_T_{b}")
                      for b in range(8)]
            with tc.tile_pool(name="psT", bufs=3, space="PSUM") as psT:
                for b in range(8):
                    tr_ps = psT.tile([128, 4 * QLOC], BF16, tag="tr")
                    for h in range(H):
                        nc.tensor.transpose(tr_ps[:, 128 * h:128 * h + 128],
                                            attn_q4[h][:, 128 * b:128 * b + 128],
                                            identb)
                    if b % 2 == 0:
                        nc.scalar.copy(attn_T[b][:], tr_ps[:])
                    else:
                        nc.vector.tensor_copy(attn_T[b][:], tr_ps[:])

            # attn_w output: sum of quarter-normalized heads (off critical path)
            av1 = sa.tile([QLOC, N], BF16)
            nc.vector.tensor_tensor(av1[:], attn_q4[0][:], attn_q4[1][:], op=OP.add)
            av2 = sa.tile([QLOC, N], BF16)
            nc.vector.tensor_tensor(av2[:], attn_q4[2][:], attn_q4[3][:], op=OP.add)
            avg = sa.tile([QLOC, N], F32)
            nc.vector.tensor_tensor(avg[:], av1[:], av2[:], op=OP.add)
            nc.gpsimd.dma_start(attn_d, avg[:])

            attendedT_c = sa.tile([D, QLOC], F32)
            with tc.tile_pool(name="psC", bufs=1, space="PSUM") as psC:
                ctxT_ps = psC.tile([D, QLOC], F32, tag="ctx")
                for b in range(8):
                    for h in range(H):
                        nc.tensor.matmul(
                            ctxT_ps[32 * h:32 * h + 32, :],
                            v_nm[:, 128 * b + 32 * h:128 * b + 32 * h + 32],
                            attn_T[b][:, 128 * h:128 * h + 128],
                            start=(b == 0), stop=(b == 7),
                            tile_position=(0, 32 * h),
                        )
                ctxT_bf = sa.tile([D, QLOC], BF16)
                nc.scalar.copy(ctxT_bf[:], ctxT_ps[:])

                att_ps = psC.tile([QLOC, D], F32, tag="att")
                nc.tensor.matmul(att_ps[:], ctxT_bf[:], woT4,
                                 start=True, stop=True)
                att_sb = sa.tile([QLOC, D], F32)
                nc.scalar.copy(att_sb[:], att_ps[:])
                attT_ps = psC.tile([D, QLOC], F32, tag="attT")
                nc.tensor.transpose(attT_ps[:], att_sb[:], identf)
                nc.scalar.copy(attendedT_c[:], attT_ps[:])

            # AllGather attendedT slices -> full attendedT [D, N]
            ag_in = dp.tile([D, QLOC], F32)
            ag_out = dp.tile([N, D], F32)
            nc.sync.dma_start(ag_in[:], attendedT_c[:])
            if sim:
                _se = [nc.sync, nc.gpsimd, nc.scalar, nc.sync]
                for c in range(NCORES):
                    _se[c % 4].dma_start(ag_out[128 * c:128 * c + 128, :],
                                         ag_in[:])
            else:
                nc.gpsimd.collective_compute(
                    "AllGather", OP.bypass,
                    replica_groups=[list(range(NCORES))],
                    ins=[ag_in.opt()], outs=[ag_out.opt()],
                )
            attendedT = sa.tile([D, N], F32)
            _eng = [nc.sync, nc.gpsimd, nc.scalar, nc.sync]
            for c in range(NCORES):
                _eng[c % 4].dma_start(attendedT[:, 128 * c:128 * c + 128],
                                      ag_out[128 * c:128 * c + 128, :])
            attendedT_bf = sa.tile([D, N], BF16)
            nc.vector.tensor_copy(attendedT_bf[:], attendedT[:])

            # n1T = W1b @ attendedT + (W1b@bprime + b1)   [2 chunks of 128 c]
            n1T_bf = [sa.tile([128, N], BF16, name=f"n1T_{ch}") for ch in range(2)]
            with tc.tile_pool(name="psN", bufs=1, space="PSUM") as psN:
                for ch in range(2):
                    n1_ps = psN.tile([128, N], F32, tag=f"n1_{ch}")
                    for nh in range(2):
                        nc.tensor.matmul(n1_ps[:, 512 * nh:512 * nh + 512],
                                         w1bT[:, 128 * ch:128 * ch + 128],
                                         attendedT_bf[:, 512 * nh:512 * nh + 512],
                                         start=True, stop=True)
                    if ch == 0:
                        nc.scalar.activation(n1T_bf[ch][:], n1_ps[:], AF.Identity,
                                             bias=bc1f[:, ch:ch + 1])
                    else:
                        nc.vector.tensor_scalar(n1T_bf[ch][:], n1_ps[:],
                                                bc1f[:, ch:ch + 1], None,
                                                op0=OP.add)

                # coordination head
                gs_sum = sa.tile([D, 1], F32)
                nc.vector.tensor_reduce(gs_sum[:], attendedT[:],
                                        axis=mybir.AxisListType.X, op=OP.add)
                gs_bf = sa.tile([D, 1], BF16)
                nc.vector.tensor_copy(gs_bf[:], gs_sum[:])
                c1_bf = sa.tile([128, 2], BF16)
                for ch in range(2):
                    c1_ps = psN.tile([128, 1], F32, tag="c1")
                    nc.tensor.matmul(c1_ps[:],
                                     wc1T[:, 128 * ch:128 * ch + 128],
                                     gs_bf[:], start=True, stop=True)
                    nc.scalar.activation(c1_bf[:, ch:ch + 1], c1_ps[:], AF.Relu,
                                         bias=bcc1[:, ch:ch + 1])
                lg_ps = psN.tile([32, 1], F32, tag="lg")
                for ch in range(2):
                    nc.tensor.matmul(lg_ps[:], wc2Ta if ch == 0 else wc2Tb,
                                     c1_bf[:, ch:ch + 1],
                                     start=(ch == 0), stop=(ch == 1))
                lg_sb = sa.tile([32, 1], F32)
                nc.scalar.activation(lg_sb[:], lg_ps[:], AF.Identity,
                                     bias=bcc2[:, 0:1])
                nc.gpsimd.dma_start(logits_d, lg_sb[:])

            # ---------- phase B: pairwise matching ----------
            # Task t: quadrant q = t % 4 (alternating for PE spread),
            # w3m column g = t // 4, score row = 32*q + g.
            with (
                tc.tile_pool(name="sbB", bufs=3) as sb,
                tc.tile_pool(name="psB", bufs=3, space="PSUM") as psB,
                tc.tile_pool(name="psSc", bufs=1, space="PSUM") as psSc,
            ):
                sc_ps = psSc.tile([128, N], F32)
                r_tiles = {}
                for run in range(TLOC // RUN):
                    # stage 2 for RUN tasks
                    for i in range(RUN):
                        t = run * RUN + i
                        h1 = [None, None]
                        for ch in range(2):
                            h1[ch] = sb.tile([128, N], BF16, tag=f"h1_{ch}",
                                             name=f"h1_{ch}_{t}")
                            nc.vector.tensor_scalar(
                                h1[ch][:], n1T_bf[ch][:],
                                t1b[:, TLOC * ch + t:TLOC * ch + t + 1], 0.0,
                                op0=OP.add, op1=OP.max)
                        g_ps = psB.tile([128, N], F32, tag="g", name=f"g_{t}")
                        for nh in range(2):
                            nc.tensor.matmul(g_ps[:, 512 * nh:512 * nh + 512],
                                             w2Ta,
                                             h1[0][:, 512 * nh:512 * nh + 512],
                                             start=True, stop=False)
                            nc.tensor.matmul(g_ps[:, 512 * nh:512 * nh + 512],
                                             w2Tb,
                                             h1[1][:, 512 * nh:512 * nh + 512],
                                             start=False, stop=True)
                        r = sb.tile([128, N], BF16, tag="r", bufs=RUN + 3,
                                    name=f"r_{t}")
                        if t % 5 == 0:
                            nc.vector.tensor_scalar(r[:], g_ps[:], b2c[:, 0:1],
                                                    0.0, op0=OP.add, op1=OP.max)
                        else:
                            nc.scalar.activation(r[:], g_ps[:], AF.Relu,
                                                 bias=b2c)
                        r_tiles[t] = r
                    # stage 3 for RUN tasks (32-col tile mode, accumulating)
                    for i in range(RUN):
                        t = run * RUN + i
                        q, g = t % 4, t // 4
                        r = r_tiles.pop(t)
                        for nh in range(2):
                            nc.tensor.matmul(
                                sc_ps[32 * q:32 * q + 32,
                                      512 * nh:512 * nh + 512],
                                w3m[:, 32 * g:32 * g + 32],
                                r[:, 512 * nh:512 * nh + 512],
                                start=(g == 0), stop=(g == 31),
                                tile_position=(0, 32 * q),
                                skip_group_check=True)
                # row 32*q + g holds task t = 4*g + q; sigmoid + out in halves
                for nh in range(2):
                    sig = sb.tile([128, 512], F32, tag="sig", bufs=2,
                                  name=f"sig_{nh}")
                    nc.scalar.activation(sig[:],
                                         sc_ps[:, 512 * nh:512 * nh + 512],
                                         AF.Sigmoid, bias=b3c)
                    _oe = [nc.sync, nc.gpsimd, nc.scalar, nc.gpsimd]
                    for q in range(4):
                        _oe[q].dma_start(
                            scores_d[q::4, 512 * nh:512 * nh + 512],
                            sig[32 * q:32 * q + 32, :])

    nc.compile()
    return nc


def _prep_inputs(inputs):
    bf = ml_dtypes.bfloat16

    def tb(a):
        return np.ascontiguousarray(a).astype(bf)

    def tf(a):
        return np.ascontiguousarray(a.astype(np.float32))

    x = np.asarray(inputs["node_states"], np.float32)[0]        # [N, D]
    task = np.asarray(inputs["task_features"], np.float32)      # [T, D]
    ipw = np.asarray(inputs["in_proj_w"], np.float32)
    ipb = np.asarray(inputs["in_proj_b"], np.float32)
    out_w = np.asarray(inputs["out_w"], np.float32)
    out_b = np.asarray(inputs["out_b"], np.float32)
    W1 = np.asarray(inputs["W1"], np.float32)
    b1 = np.asarray(inputs["b1"], np.float32)
    W2 = np.asarray(inputs["W2"], np.float32)
    b2 = np.asarray(inputs["b2"], np.float32)
    W3 = np.asarray(inputs["W3"], np.float32)
    b3 = np.asarray(inputs["b3"], np.float32)
    Wc1 = np.asarray(inputs["Wc1"], np.float32)
    bc1 = np.asarray(inputs["bc1"], np.float32)
    Wc2 = np.asarray(inputs["Wc2"], np.float32)
    bc2 = np.asarray(inputs["bc2"], np.float32)

    xT = x.T                                                    # [D, N]
    taskT = task.T                                              # [D, T]
    bprime = out_w @ ipb[256:384] + out_b                       # [128]
    bc1f = (W1[:, 128:] @ bprime + b1).reshape(2, 128).T        # [128, 2]
    bcc1 = (Wc1 @ bprime + bc1).reshape(2, 128).T               # [128, 2]
    w3m = np.zeros((128, 32 * 32), np.float32)
    for g in range(32):
        w3m[:, 32 * g + g] = W3[0]
    ident = np.eye(128, dtype=np.float32)

    wpack = np.concatenate([
        ipw[0:128].T,            # wqT     0:128
        ipw[128:256].T,          # wkT     128:256
        ipw[256:384].T,          # wvT     256:384
        out_w.T * 4.0,           # woT4    384:512
        W1[:, :128].T,           # w1aT    512:768
        W1[:, 128:].T,           # w1bT    768:1024
        W2[:, :128].T,           # w2Ta    1024:1152
        W2[:, 128:].T,           # w2Tb    1152:1280
        Wc1.T / 1024.0,          # wc1T    1280:1536
        Wc2[:, :128].T,          # wc2Ta   1536:1568
        Wc2[:, 128:].T,          # wc2Tb   1568:1600
        ident,                   # identb  1600:1728
        w3m,                     # w3m     1728:2752
    ], axis=1)
    bpack = np.concatenate([
        (ipb[0:128] / math.sqrt(DH))[:, None],   # bqs   0
        ipb[128:256][:, None],                   # bk    1
        bc1f,                                    # bc1f  2:4
        b2[:, None],                             # b2c   4
        np.full((128, 1), b3[0], np.float32),    # b3c   5
        bcc1,                                    # bcc1  6:8
        ident,                                   # identf 8:136
    ], axis=1)

    shared = {
        "xT": tb(xT),
        "wpack": tb(wpack),
        "bpack": tf(bpack),
        "bcc2": tf(bc2[:, None]),
    }
    in_maps = []
    for c in range(NCORES):
        m = dict(shared)
        m["xTq"] = tb(xT[:, QLOC * c:QLOC * c + QLOC])
        m["taskT"] = tb(taskT[:, TLOC * c:TLOC * c + TLOC])
        in_maps.append(m)
    return in_maps


def kernel(**inputs):
    if "nc" not in _CACHE:
        _CACHE["nc"] = _build_program()
    nc = _CACHE["nc"]
    in_maps = _prep_inputs(inputs)
    res = run_bass_kernel_spmd(nc, in_maps, list(range(NCORES))).results

    # scores row 32*q + g holds task 4*g + q; the strided DMA already
    # restored task order into scores_d rows q::4 <- rows 32q..32q+32.
    matching = np.concatenate([res[c]["scores"] for c in range(NCORES)], axis=0)
    attn_w = np.concatenate([res[c]["attn"] for c in range(NCORES)], axis=0)[None]
    logits = res[0]["logits"].reshape(1, 32)
    return (matching.astype(np.float32), logits.astype(np.float32),
            attn_w.astype(np.float32))


# revision 14
# speedup vs baseline: 1.0679x; 1.0465x over previous
"""Trainium2 Bass kernel for nn_ClusterPolicyNetwork.

Computation (reference.py):
  1. 4-head self-attention over N=1024 nodes (D=128), returns attended nodes
     and head-averaged attention map attn_w [1,1024,1024].
  2. Pairwise matching MLP over (task, node) grid:
     h1 = relu(t1[t] + n1[n] + b1)  (256)
     h2 = relu(W2 h1 + b2)          (128)
     score = sigmoid(W3 h2 + b3)    -> [1024, 1024]
  3. Coordination head on mean of attended -> [1, 32].

Sharding: tasks and attention q-rows both split 128/core across 8 cores.
Each core computes attention for its q-slice; an AllGather shares the
attended nodes; each core then computes a [128, 1024] score tile.

Layouts are "transposed" (feature on partitions) throughout. All biases are
folded into evacuation ops or host-precomputed weight/bias transforms.
bf16 matmul operands, fp32 accumulation.

Stage-3 trick: the W3 dot for task t uses a stationary [128, 32] matrix with
W3 in column t%32 and zeros elsewhere, accumulated (start/stop per quadrant
column-group) into one persistent PSUM tile so that task t's score row lands
at partition 32*(t%4) + t//4... (see code: row = 32*quad + col), giving a
single sigmoid evacuation for all 128 tasks.
"""

import math

import numpy as np
import ml_dtypes

from concourse import mybir
import concourse.bacc as bacc
import concourse.tile as tile
from concourse.bass_utils import run_bass_kernel_spmd

F32 = mybir.dt.float32
BF16 = mybir.dt.bfloat16
AF = mybir.ActivationFunctionType
OP = mybir.AluOpType

NCORES = 8
N = 1024          # nodes
T = 1024          # tasks
D = 128           # d_model
H = 4             # heads
DH = 32           # head dim
TLOC = T // NCORES
QLOC = N // NCORES
RUN = 8           # tasks per stage2/stage3 batch (PE mode-switch batching)
WPACK_COLS = 2752  # packed bf16 weights (see wsl() offsets)
BPACK_COLS = 456   # packed f32 bias columns + identity + coord weights

_CACHE = {}


def _build_program(sim=False):
    # sim=True builds a single-core variant with the collective replaced by
    # local DMAs, for cost-model timeline simulation only.
    nc = bacc.Bacc("TRN2", target_bir_lowering=False, debug=False,
                   enable_asserts=True, num_devices=1 if sim else NCORES)

    def din(name, shape, dt=BF16):
        return nc.dram_tensor(name, shape, dt, kind="ExternalInput").ap()

    def dout(name, shape, dt=F32):
        return nc.dram_tensor(name, shape, dt, kind="ExternalOutput").ap()

    # inputs (host-prepped layouts).  All bf16 weights ride in one packed
    # tensor (single DMA); all f32 bias columns + identity in another.
    xT_d = din("xT", [D, N])                 # node_states[0].T      (shared)
    xTq_d = din("xTq", [D, QLOC])            # q-slice of xT         (per core)
    taskT_d = din("taskT", [D, TLOC])        # task slice, transposed (per core)
    wpack_d = din("wpack", [128, WPACK_COLS])
    bpack_d = din("bpack", [128, BPACK_COLS], F32)
    bcc2_d = din("bcc2", [32, 1], F32)

    scores_d = dout("scores", [TLOC, N])
    attn_d = dout("attn", [QLOC, N])
    logits_d = dout("logits", [32, 1])

    SQS = 1.0 / math.sqrt(DH)

    with tile.TileContext(nc) as tc:
        with (
            tc.tile_pool(name="wpool", bufs=1) as wp,
            tc.tile_pool(name="sbA", bufs=1) as sa,
            tc.tile_pool(name="dramp", bufs=1, space="DRAM") as dp,
        ):
            # --- load constants (few big DMAs, spread over engine queues) ---
            xTq = wp.tile([D, QLOC], BF16)
            nc.sync.dma_start(xTq[:], xTq_d)
            xT = wp.tile([D, N], BF16)
            nc.sync.dma_start(xT[:], xT_d)
            wpack = wp.tile([128, WPACK_COLS], BF16)
            nc.gpsimd.dma_start(wpack[:], wpack_d)
            bpack = wp.tile([128, BPACK_COLS], F32)
            nc.scalar.dma_start(bpack[:], bpack_d)
            taskT = wp.tile([D, TLOC], BF16)
            nc.scalar.dma_start(taskT[:], taskT_d)
            bcc2 = wp.tile([32, 1], F32)
            nc.scalar.dma_start(bcc2[:], bcc2_d)

            def wsl(a, b):
                return wpack[:, a:b]

            wqT = wsl(0, 128)
            wkT = wsl(128, 256)
            wvT = wsl(256, 384)
            woT4 = wsl(384, 512)
            w1aT = wsl(512, 768)
            w1bT = wsl(768, 1024)
            w2Ta = wsl(1024, 1152)
            w2Tb = wsl(1152, 1280)
            wc1T = wsl(1280, 1536)
            wc2Ta = wsl(1536, 1568)
            wc2Tb = wsl(1568, 1600)
            identb = wsl(1600, 1728)
            w3m = wsl(1728, 2752)
            bqs = bpack[:, 0:1]
            bk = bpack[:, 1:2]
            bc1f = bpack[:, 2:4]
            b2c = bpack[:, 4:5]
            b3c = bpack[:, 5:6]
            bcc1 = bpack[:, 6:8]
            identf = bpack[:, 8:136]
            wc1T_f = bpack[:, 136:392]
            wc2a_f = bpack[:, 392:424]
            wc2b_f = bpack[:, 424:456]

            # ---------- phase A: attention on the q-slice ----------
            qT_bf = sa.tile([D, QLOC], BF16)
            kT_bf = sa.tile([D, N], BF16)
            v_nm = sa.tile([128, N], BF16)     # [n_in_block, 128*b + d']
            t1b = sa.tile([128, 2 * TLOC], F32)  # per-task bias cols (2 chunks)

            attn_q4 = [sa.tile([QLOC, N], BF16, name=f"attn_q4_{h}")
                       for h in range(H)]
            with tc.tile_pool(name="psA", bufs=1, space="PSUM") as psA:
                qT_ps = psA.tile([D, QLOC], F32, tag="q")
                nc.tensor.matmul(qT_ps[:], wqT, xTq[:], start=True, stop=True)
                nc.scalar.activation(qT_bf[:], qT_ps[:], AF.Identity,
                                     bias=bqs, scale=SQS)

                kT_ps = psA.tile([D, N], F32, tag="k")
                for nh in range(2):
                    nc.tensor.matmul(kT_ps[:, 512 * nh:512 * nh + 512], wkT,
                                     xT[:, 512 * nh:512 * nh + 512],
                                     start=True, stop=True)
                nc.vector.tensor_scalar(kT_bf[:], kT_ps[:], bk, None,
                                        op0=OP.add)

                # softmax (no max subtraction: logits are O(1))
                for h in range(H):
                    s_ps = psA.tile([QLOC, N], F32, tag="s", bufs=2,
                                    name=f"s_ps_{h}")
                    for kh in range(2):
                        nc.tensor.matmul(s_ps[:, 512 * kh:512 * kh + 512],
                                         qT_bf[32 * h:32 * h + 32, :],
                                         kT_bf[32 * h:32 * h + 32,
                                               512 * kh:512 * kh + 512],
                                         start=True, stop=True,
                                         tile_position=(32 * h, 0))
                    exp_h = sa.tile([QLOC, N], F32, tag="exp", bufs=2,
                                    name=f"exp_{h}")
                    sumexp = sa.tile([QLOC, 1], F32, tag="sumexp", bufs=4,
                                     name=f"sumexp_{h}")
                    nc.scalar.activation(exp_h[:], s_ps[:], AF.Exp,
                                         accum_out=sumexp[:])
                    se4 = sa.tile([QLOC, 1], F32, tag="se4", bufs=4,
                                  name=f"se4_{h}")
                    nc.vector.tensor_scalar_mul(se4[:], sumexp[:], 4.0)
                    rec4 = sa.tile([QLOC, 1], F32, tag="rec4", bufs=4,
                                   name=f"rec4_{h}")
                    nc.vector.reciprocal(rec4[:], se4[:])
                    nc.vector.tensor_scalar_mul(attn_q4[h][:], exp_h[:],
                                                rec4[:, 0:1])

                # v projection (n-major) and t1, reusing the k/q psum slots
                v_ps = psA.tile([128, N], F32, tag="k", name="v_ps")
                for b in range(8):
                    nc.tensor.matmul(v_ps[:, 128 * b:128 * b + 128],
                                     xT[:, 128 * b:128 * b + 128], wvT,
                                     start=True, stop=True)
                nc.vector.tensor_copy(v_nm[:], v_ps[:])

                t1_ps = psA.tile([128, 2 * TLOC], F32, tag="q", name="t1_ps")
                for ch in range(2):
                    nc.tensor.matmul(t1_ps[:, TLOC * ch:TLOC * ch + TLOC],
                                     w1aT[:, 128 * ch:128 * ch + 128], taskT[:],
                                     start=True, stop=True)
                nc.scalar.copy(t1b[:], t1_ps[:])

            # transpose attn (per 128-node block) and apply attention to v
            attn_T = [sa.tile([128, 4 * QLOC], BF16, name=f"attn_T_{b}")
                      for b in range(8)]
            with tc.tile_pool(name="psT", bufs=3, space="PSUM") as psT:
                for b in range(8):
                    tr_ps = psT.tile([128, 4 * QLOC], BF16, tag="tr")
                    for h in range(H):
                        nc.tensor.transpose(tr_ps[:, 128 * h:128 * h + 128],
                                            attn_q4[h][:, 128 * b:128 * b + 128],
                                            identb)
                    if b % 2 == 0:
                        nc.scalar.copy(attn_T[b][:], tr_ps[:])
                    else:
                        nc.vector.tensor_copy(attn_T[b][:], tr_ps[:])

            # attn_w output: sum of quarter-normalized heads (off critical path)
            av1 = sa.tile([QLOC, N], BF16)
            nc.vector.tensor_tensor(av1[:], attn_q4[0][:], attn_q4[1][:], op=OP.add)
            av2 = sa.tile([QLOC, N], BF16)
            nc.vector.tensor_tensor(av2[:], attn_q4[2][:], attn_q4[3][:], op=OP.add)
            avg = sa.tile([QLOC, N], F32)
            nc.vector.tensor_tensor(avg[:], av1[:], av2[:], op=OP.add)
            nc.gpsimd.dma_start(attn_d, avg[:])

            attendedT_c = sa.tile([D, QLOC], F32)
            with tc.tile_pool(name="psC", bufs=1, space="PSUM") as psC:
                ctxT_ps = psC.tile([D, QLOC], F32, tag="ctx")
                for b in range(8):
                    for h in range(H):
                        nc.tensor.matmul(
                            ctxT_ps[32 * h:32 * h + 32, :],
                            v_nm[:, 128 * b + 32 * h:128 * b + 32 * h + 32],
                            attn_T[b][:, 128 * h:128 * h + 128],
                            start=(b == 0), stop=(b == 7),
                            tile_position=(0, 32 * h),
                        )
                ctxT_bf = sa.tile([D, QLOC], BF16)
                nc.scalar.copy(ctxT_bf[:], ctxT_ps[:])

                att_ps = psC.tile([QLOC, D], F32, tag="att")
                nc.tensor.matmul(att_ps[:], ctxT_bf[:], woT4,
                                 start=True, stop=True)
                att_sb = sa.tile([QLOC, D], F32)
                nc.scalar.copy(att_sb[:], att_ps[:])
                attT_ps = psC.tile([D, QLOC], F32, tag="attT")
                nc.tensor.transpose(attT_ps[:], att_sb[:], identf)
                nc.scalar.copy(attendedT_c[:], attT_ps[:])

            # AllGather attendedT slices -> full attendedT [D, N]
            ag_in = dp.tile([D, QLOC], F32)
            ag_out = dp.tile([N, D], F32)
            nc.sync.dma_start(ag_in[:], attendedT_c[:])
            if sim:
                _se = [nc.sync, nc.gpsimd, nc.scalar, nc.sync]
                for c in range(NCORES):
                    _se[c % 4].dma_start(ag_out[128 * c:128 * c + 128, :],
                                         ag_in[:])
            else:
                nc.gpsimd.collective_compute(
                    "AllGather", OP.bypass,
                    replica_groups=[list(range(NCORES))],
                    ins=[ag_in.opt()], outs=[ag_out.opt()],
                )
            attendedT = sa.tile([D, N], F32)
            _eng = [nc.sync, nc.gpsimd, nc.scalar, nc.sync]
            for c in range(NCORES):
                _eng[c % 4].dma_start(attendedT[:, 128 * c:128 * c + 128],
                                      ag_out[128 * c:128 * c + 128, :])
            attendedT_bf = sa.tile([D, N], BF16)
            nc.vector.tensor_copy(attendedT_bf[:], attendedT[:])

            # n1T = W1b @ attendedT + (W1b@bprime + b1)   [2 chunks of 128 c]
            n1T_bf = [sa.tile([128, N], BF16, name=f"n1T_{ch}") for ch in range(2)]
            with tc.tile_pool(name="psN", bufs=1, space="PSUM") as psN:
                for ch in range(2):
                    n1_ps = psN.tile([128, N], F32, tag=f"n1_{ch}")
                    for nh in range(2):
                        nc.tensor.matmul(n1_ps[:, 512 * nh:512 * nh + 512],
                                         w1bT[:, 128 * ch:128 * ch + 128],
                                         attendedT_bf[:, 512 * nh:512 * nh + 512],
                                         start=True, stop=True)
                    if ch == 0:
                        nc.scalar.activation(n1T_bf[ch][:], n1_ps[:], AF.Identity,
                                             bias=bc1f[:, ch:ch + 1])
                    else:
                        nc.vector.tensor_scalar(n1T_bf[ch][:], n1_ps[:],
                                                bc1f[:, ch:ch + 1], None,
                                                op0=OP.add)

                # coordination head (plain fp32: N=1 matmuls, cost-free)
                gs_sum = sa.tile([D, 1], F32)
                nc.vector.tensor_reduce(gs_sum[:], attendedT[:],
                                        axis=mybir.AxisListType.X, op=OP.add)
                c1_f = sa.tile([128, 2], F32)
                for ch in range(2):
                    c1_ps = psN.tile([128, 1], F32, tag="c1")
                    nc.tensor.matmul(c1_ps[:],
                                     wc1T_f[:, 128 * ch:128 * ch + 128],
                                     gs_sum[:], start=True, stop=True)
                    nc.scalar.activation(c1_f[:, ch:ch + 1], c1_ps[:], AF.Relu,
                                         bias=bcc1[:, ch:ch + 1])
                lg_ps = psN.tile([32, 1], F32, tag="lg")
                for ch in range(2):
                    nc.tensor.matmul(lg_ps[:], wc2a_f if ch == 0 else wc2b_f,
                                     c1_f[:, ch:ch + 1],
                                     start=(ch == 0), stop=(ch == 1))
                lg_sb = sa.tile([32, 1], F32)
                nc.scalar.activation(lg_sb[:], lg_ps[:], AF.Identity,
                                     bias=bcc2[:, 0:1])
                nc.gpsimd.dma_start(logits_d, lg_sb[:])

            # ---------- phase B: pairwise matching ----------
            # Task t: quadrant q = t % 4 (alternating for PE spread),
            # w3m column g = t // 4, score row = 32*q + g.
            with (
                tc.tile_pool(name="sbB", bufs=3) as sb,
                tc.tile_pool(name="psB", bufs=3, space="PSUM") as psB,
                tc.tile_pool(name="psSc", bufs=1, space="PSUM") as psSc,
            ):
                sc_ps = psSc.tile([128, N], F32)
                r_tiles = {}
                for run in range(TLOC // RUN):
                    # stage 2 for RUN tasks
                    for i in range(RUN):
                        t = run * RUN + i
                        h1 = [None, None]
                        for ch in range(2):
                            h1[ch] = sb.tile([128, N], BF16, tag=f"h1_{ch}",
                                             name=f"h1_{ch}_{t}")
                            nc.vector.tensor_scalar(
                                h1[ch][:], n1T_bf[ch][:],
                                t1b[:, TLOC * ch + t:TLOC * ch + t + 1], 0.0,
                                op0=OP.add, op1=OP.max)
                        g_ps = psB.tile([128, N], F32, tag="g", name=f"g_{t}")
                        for nh in range(2):
                            nc.tensor.matmul(g_ps[:, 512 * nh:512 * nh + 512],
                                             w2Ta,
                                             h1[0][:, 512 * nh:512 * nh + 512],
                                             start=True, stop=False)
                            nc.tensor.matmul(g_ps[:, 512 * nh:512 * nh + 512],
                                             w2Tb,
                                             h1[1][:, 512 * nh:512 * nh + 512],
                                             start=False, stop=True)
                        r = sb.tile([128, N], BF16, tag="r", bufs=RUN + 3,
                                    name=f"r_{t}")
                        if t % 5 == 0:
                            nc.vector.tensor_scalar(r[:], g_ps[:], b2c[:, 0:1],
                                                    0.0, op0=OP.add, op1=OP.max)
                        else:
                            nc.scalar.activation(r[:], g_ps[:], AF.Relu,
                                                 bias=b2c)
                        r_tiles[t] = r
                    # stage 3 for RUN tasks (32-col tile mode, accumulating)
                    for i in range(RUN):
                        t = run * RUN + i
                        q, g = t % 4, t // 4
                        r = r_tiles.pop(t)
                        for nh in range(2):
                            nc.tensor.matmul(
                                sc_ps[32 * q:32 * q + 32,
                                      512 * nh:512 * nh + 512],
                                w3m[:, 32 * g:32 * g + 32],
                                r[:, 512 * nh:512 * nh + 512],
                                start=(g == 0), stop=(g == 31),
                                tile_position=(0, 32 * q),
                                skip_group_check=True)
                # row 32*q + g holds task t = 4*g + q; sigmoid + out in halves
                for nh in range(2):
                    sig = sb.tile([128, 512], F32, tag="sig", bufs=2,
                                  name=f"sig_{nh}")
                    nc.scalar.activation(sig[:],
                                         sc_ps[:, 512 * nh:512 * nh + 512],
                                         AF.Sigmoid, bias=b3c)
                    _oe = [nc.sync, nc.gpsimd, nc.scalar, nc.gpsimd]
                    for q in range(4):
                        _oe[q].dma_start(
                            scores_d[q::4, 512 * nh:512 * nh + 512],
                            sig[32 * q:32 * q + 32, :])

    nc.compile()
    return nc


def _prep_inputs(inputs):
    bf = ml_dtypes.bfloat16

    def tb(a):
        return np.ascontiguousarray(a).astype(bf)

    def tf(a):
        return np.ascontiguousarray(a.astype(np.float32))

    x = np.asarray(inputs["node_states"], np.float32)[0]        # [N, D]
    task = np.asarray(inputs["task_features"], np.float32)      # [T, D]
    ipw = np.asarray(inputs["in_proj_w"], np.float32)
    ipb = np.asarray(inputs["in_proj_b"], np.float32)
    out_w = np.asarray(inputs["out_w"], np.float32)
    out_b = np.asarray(inputs["out_b"], np.float32)
    W1 = np.asarray(inputs["W1"], np.float32)
    b1 = np.asarray(inputs["b1"], np.float32)
    W2 = np.asarray(inputs["W2"], np.float32)
    b2 = np.asarray(inputs["b2"], np.float32)
    W3 = np.asarray(inputs["W3"], np.float32)
    b3 = np.asarray(inputs["b3"], np.float32)
    Wc1 = np.asarray(inputs["Wc1"], np.float32)
    bc1 = np.asarray(inputs["bc1"], np.float32)
    Wc2 = np.asarray(inputs["Wc2"], np.float32)
    bc2 = np.asarray(inputs["bc2"], np.float32)

    xT = x.T                                                    # [D, N]
    taskT = task.T                                              # [D, T]
    bprime = out_w @ ipb[256:384] + out_b                       # [128]
    bc1f = (W1[:, 128:] @ bprime + b1).reshape(2, 128).T        # [128, 2]
    bcc1 = (Wc1 @ bprime + bc1).reshape(2, 128).T               # [128, 2]
    w3m = np.zeros((128, 32 * 32), np.float32)
    for g in range(32):
        w3m[:, 32 * g + g] = W3[0]
    ident = np.eye(128, dtype=np.float32)

    wpack = np.concatenate([
        ipw[0:128].T,            # wqT     0:128
        ipw[128:256].T,          # wkT     128:256
        ipw[256:384].T,          # wvT     256:384
        out_w.T * 4.0,           # woT4    384:512
        W1[:, :128].T,           # w1aT    512:768
        W1[:, 128:].T,           # w1bT    768:1024
        W2[:, :128].T,           # w2Ta    1024:1152
        W2[:, 128:].T,           # w2Tb    1152:1280
        Wc1.T / 1024.0,          # wc1T    1280:1536
        Wc2[:, :128].T,          # wc2Ta   1536:1568
        Wc2[:, 128:].T,          # wc2Tb   1568:1600
        ident,                   # identb  1600:1728
        w3m,                     # w3m     1728:2752
    ], axis=1)
    bpack = np.concatenate([
        (ipb[0:128] / math.sqrt(DH))[:, None],   # bqs   0
        ipb[128:256][:, None],                   # bk    1
        bc1f,                                    # bc1f  2:4
        b2[:, None],                             # b2c   4
        np.full((128, 1), b3[0], np.float32),    # b3c   5
        bcc1,                                    # bcc1  6:8
        ident,                                   # identf 8:136
        Wc1.T / 1024.0,                          # wc1T_f 136:392
        Wc2[:, :128].T,                          # wc2a_f 392:424
        Wc2[:, 128:].T,                          # wc2b_f 424:456
    ], axis=1)

    shared = {
        "xT": tb(xT),
        "wpack": tb(wpack),
        "bpack": tf(bpack),
        "bcc2": tf(bc2[:, None]),
    }
    in_maps = []
    for c in range(NCORES):
        m = dict(shared)
        m["xTq"] = tb(xT[:, QLOC * c:QLOC * c + QLOC])
        m["taskT"] = tb(taskT[:, TLOC * c:TLOC * c + TLOC])
        in_maps.append(m)
    return in_maps


def kernel(**inputs):
    if "nc" not in _CACHE:
        _CACHE["nc"] = _build_program()
    nc = _CACHE["nc"]
    in_maps = _prep_inputs(inputs)
    res = run_bass_kernel_spmd(nc, in_maps, list(range(NCORES))).results

    # scores row 32*q + g holds task 4*g + q; the strided DMA already
    # restored task order into scores_d rows q::4 <- rows 32q..32q+32.
    matching = np.concatenate([res[c]["scores"] for c in range(NCORES)], axis=0)
    attn_w = np.concatenate([res[c]["attn"] for c in range(NCORES)], axis=0)[None]
    logits = res[0]["logits"].reshape(1, 32)
    return (matching.astype(np.float32), logits.astype(np.float32),
            attn_w.astype(np.float32))
